# revision 16
# baseline (speedup 1.0000x reference)
"""Trainium2 Bass kernel for nn_Encoder_Model_15874199126585 (align-loss).

loss = mean_i[ lse_l(i) + lse_r(i) ] where, per side,
  x[i,j] = pos[i] - (||A_i||^2 + ||e_j||^2 - 2 A_i.e_j) + GAMMA
  y      = x * mask          (mask kills cols l_i, r_i)
  lse    = logsumexp(LAMB*(y-mu)/sd + TAU, axis=-1)

Strategy (8 NeuronCores, emb rows N-sharded 12500/core, no collectives):
 * mean/std per row are computed on HOST in f64 closed form (Gram-matrix
   quadratic forms), so the device needs no stats passes or collectives.
 * each core computes its [B, 12800(padded)] slice of x'' = A.e_j + cc_j/2
   (cc_j = -||e_j||^2): per 512-col chunk, 4 fp8 matmuls accumulate the
   dot in PSUM, then a k=2 bf16 matmul (ones  @ [cc_hi; cc_lo]) adds the
   per-column constant, so no DVE pass and no replicated cc upload.
 * the "self" column (j == own index, value pos+GAMMA, which would dominate
   the softmax) is killed inside PSUM by one more accumulating matmul:
   (-1e30*I).T @ onehot, where the host permutation placed every column that
   can ever be a self column into chunk 0 ("hot block").  The onehot rhs and
   the -1e30*I weights are built ON DEVICE from a 16KB index upload
   (iota + is_equal), not uploaded.
 * the gathered pair-row tiles (lt/rt) are NOT replicated to all 8 cores:
   each core uploads 2 of the 16 row tiles (262KB) and a device AllGather
   (DRAM bounce buffers, replica group 0-7) reconstructs the full set.
 * because rows are exactly normalized, z = LAMB*(x-mu)/sd + TAU lies in a
   known narrow band, so a FIXED stabilizer M0 replaces the usual row-max:
   one fused ACT pass per chunk computes exp(psum*(2a) + bias) with bias =
   a*(rc-mu)+TAU-M0 precomputed on host (rc = pos - ||A||^2 + GAMMA), and
   its accum_out gives the chunk row-sum for free.
 * device emits per-(row, tile, side, chunk) partial sums S; host does the
   log-sum-exp combine in f64 and adds the analytic contribution of the
   masked-out entries.

All tensor inputs ride to the device in fp8e4m3 (emb values are N(0,1),
max |v| ~ 5, well inside e4m3 range); the exact per-row normalization and
the final f64 combine keep the end-to-end relative error ~1e-3, far inside
the 2e-2 gate, while cutting per-call PJRT upload from ~225MB to ~55MB --
the axon-tunnel transfer is what dominates wall time, not device compute
(~1ms).  A persistent XLA compilation cache removes the ~1s/call re-verify
that run_bass_kernel_spmd's fresh-jit-per-call pattern otherwise incurs.
"""

import os
import sys
from contextlib import ExitStack

import numpy as np

sys.path.insert(0, "/opt/trn_rl_repo")

import ml_dtypes

NODE = 100000
DIM = 512
B = 2048
GAMMA, LAMB, TAU = 3.0, 20.0, 8.0
NCORES = 8
CHUNK = 512
NCHUNK = 25
NS_PAD = NCHUNK * CHUNK          # 12800 DRAM-layout columns per core
LAST_W = 256                     # last chunk is trimmed to 256 columns
NS_USED = (NCHUNK - 1) * CHUNK + LAST_W  # 12544 columns actually computed
NS_REAL = NODE // NCORES         # 12500
HOT = 512                        # hot block = chunk 0 (all possible self cols)
NT = B // 128                    # 16 row tiles
NEG_BIG = -1.0e30
M0 = 100.0                       # fixed logsumexp stabilizer (z in [~84, ~110])
TPC = NT // NCORES               # pair row tiles uploaded per core (AllGathered)

FP8 = ml_dtypes.float8_e4m3      # TRN2 float8e4


# --------------------------------------------------------------------------
# host-side preparation
# --------------------------------------------------------------------------

def _host_prepare(pairs, emb):
    pairs = np.asarray(pairs)
    emb = np.asarray(emb, dtype=np.float32)
    l = pairs[:, 0].astype(np.int64)
    r = pairs[:, 1].astype(np.int64)
    emb64 = emb.astype(np.float64)

    l_emb = emb[l]
    r_emb = emb[r]
    l64, r64 = emb64[l], emb64[r]

    emb_sq64 = np.sum(emb64 * emb64, axis=1)
    pos64 = np.sum((l64 - r64) ** 2, axis=1)
    a_sq64 = emb_sq64[l]
    b_sq64 = emb_sq64[r]
    cc64 = -emb_sq64

    rc_l = pos64 - a_sq64 + GAMMA
    rc_r = pos64 - b_sq64 + GAMMA

    s_vec = emb64.sum(axis=0)
    w_vec = (emb64 * cc64[:, None]).sum(axis=0)
    C1 = cc64.sum()
    C2 = (cc64 * cc64).sum()
    try:
        from scipy.linalg.blas import dsyrk
        G = dsyrk(1.0, emb64, trans=1)       # upper triangle of emb64.T@emb64
        G = np.triu(G) + np.triu(G, 1).T
    except Exception:
        G = emb64.T @ emb64

    def side_stats(A64, rc):
        As = A64 @ s_vec
        Aw = A64 @ w_vec
        qf = np.einsum("bd,bd->b", A64 @ G, A64)
        S1 = 2.0 * As + NODE * rc + C1
        S2 = (4.0 * qf + 4.0 * Aw + 4.0 * rc * As + NODE * rc * rc
              + 2.0 * rc * C1 + C2)
        return S1, S2

    S1_l, S2_l = side_stats(l64, rc_l)
    S1_r, S2_r = side_stats(r64, rc_r)

    dot_lr = np.einsum("bd,bd->b", l64, r64)
    x_self_l = 2.0 * a_sq64 + rc_l + cc64[l]
    x_cross_l = 2.0 * dot_lr + rc_l + cc64[r]
    x_self_r = 2.0 * b_sq64 + rc_r + cc64[r]
    x_cross_r = 2.0 * dot_lr + rc_r + cc64[l]

    eq = l == r

    def masked_stats(S1, S2, x_self, x_cross):
        S1m = np.where(eq, S1 - 2.0 * x_self, S1 - x_self - x_cross)
        S2m = np.where(eq, S2, S2 - x_self ** 2 - x_cross ** 2)
        mu = S1m / NODE
        var = S2m / NODE - mu * mu
        sd = np.sqrt(var)
        return mu, sd

    mu_l, sd_l = masked_stats(S1_l, S2_l, x_self_l, x_cross_l)
    mu_r, sd_r = masked_stats(S1_r, S2_r, x_self_r, x_cross_r)

    # core assignment: every value appearing in pairs goes into some core's
    # 512-column hot block (front of its local column range)
    hot = np.unique(np.concatenate([l, r]))
    hot_per_core = [hot[c::NCORES] for c in range(NCORES)]
    for c in range(NCORES):
        assert len(hot_per_core[c]) <= HOT - 1, (c, len(hot_per_core[c]))
    cold_mask = np.ones(NODE, dtype=bool)
    cold_mask[hot] = False
    cold = np.nonzero(cold_mask)[0]

    bf16 = ml_dtypes.bfloat16
    cores = []
    off = 0
    for c in range(NCORES):
        nh = len(hot_per_core[c])
        need = NS_REAL - nh
        cold_c = cold[off:off + need]
        off += need
        colmap = np.full(NS_PAD, -1, dtype=np.int64)
        colmap[:nh] = hot_per_core[c]
        assert HOT + need <= NS_USED
        colmap[HOT:HOT + need] = cold_c
        valid = colmap >= 0

        embT = np.zeros((DIM, NS_PAD), dtype=np.float32)
        embT[:, valid] = emb[colmap[valid]].T
        cch64 = np.full(NS_PAD, NEG_BIG / 2, dtype=np.float64)
        cch64[valid] = cc64[colmap[valid]] / 2.0

        g2loc = {int(colmap[j]): j for j in range(nh)}
        padcol = HOT - 1
        assert colmap[padcol] == -1
        w_l = np.array([g2loc.get(int(v), padcol) for v in l], dtype=np.int64)
        w_r = np.array([g2loc.get(int(v), padcol) for v in r], dtype=np.int64)

        # device input layouts
        # embt: [128(k), NCHUNK, 4(d), 512(n)] fp8
        embt_dev = np.ascontiguousarray(
            embT.astype(FP8)
            .reshape(4, 128, NCHUNK, CHUNK)
            .transpose(1, 2, 0, 3)
        )
        # cc/2 split hi/lo in bf16: [2, NCHUNK, 512]
        hi = cch64.astype(bf16)
        lo = (cch64 - hi.astype(np.float64)).astype(bf16)
        cchl_dev = np.ascontiguousarray(
            np.stack([hi, lo], axis=0).reshape(2, NCHUNK, CHUNK))
        # self-suppression one-hot column index per (row-in-tile, tile, side)
        wsel = np.stack([w_l.reshape(NT, 128), w_r.reshape(NT, 128)], axis=-1)
        wsel_dev = np.ascontiguousarray(
            wsel.transpose(1, 0, 2).astype(np.float32))     # [128, NT, 2]
        cores.append(dict(embt=embt_dev, cchl=cchl_dev, wsel=wsel_dev))
    assert off == len(cold)

    # pair-row tiles, B-sharded across cores: core c uploads row tiles
    # {2c, 2c+1} of both sides and the device AllGathers the full set.
    def tile_A(A):
        # A [B, D] f32 -> [NT, 128(k), 4(d), 128(m)] fp8 of A^T
        At = A.T.astype(FP8)                      # [D, B]
        return np.ascontiguousarray(
            At.reshape(4, 128, NT, 128).transpose(2, 1, 0, 3))

    lt_dev = tile_A(l_emb)
    rt_dev = tile_A(r_emb)
    TPC = NT // NCORES               # row tiles per core
    ab_shards = []
    for c in range(NCORES):
        sh = np.stack([lt_dev[c * TPC:(c + 1) * TPC],
                       rt_dev[c * TPC:(c + 1) * TPC]], axis=0)
        ab_shards.append(np.ascontiguousarray(sh))  # [2, TPC, 128, 4, 128]

    alpha_l = LAMB / sd_l
    alpha_r = LAMB / sd_r
    scale2a = np.stack([2.0 * alpha_l, 2.0 * alpha_r], axis=-1)
    biash0 = np.stack([alpha_l * (rc_l - mu_l) + TAU,
                       alpha_r * (rc_r - mu_r) + TAU], axis=-1)
    scale2a_dev = np.ascontiguousarray(
        scale2a.reshape(NT, 128, 2).transpose(1, 0, 2)).astype(np.float32)
    biash0_dev = np.ascontiguousarray(
        biash0.reshape(NT, 128, 2).transpose(1, 0, 2))
    host = dict(
        eq=eq, mu_l=mu_l, sd_l=sd_l, mu_r=mu_r, sd_r=sd_r,
        x_self_l=x_self_l, x_self_r=x_self_r,
        cores=cores, ab_shards=ab_shards,
        scale2a=scale2a_dev, biash0=biash0_dev,
    )
    return host


def _make_in_maps(host, m0):
    biash = (host["biash0"] - m0).astype(np.float32)
    in_maps = []
    for c in range(NCORES):
        core = host["cores"][c]
        in_maps.append(dict(
            embt=core["embt"], ab=host["ab_shards"][c],
            cchl=core["cchl"], wsel=core["wsel"],
            scale2a=host["scale2a"], biash=biash,
        ))
    return in_maps


# --------------------------------------------------------------------------
# bass kernel
# --------------------------------------------------------------------------

def _build_bass():
    import concourse.mybir as mybir
    import concourse.tile as tile
    from concourse import bacc

    P = 128
    f32 = mybir.dt.float32
    bf = mybir.dt.bfloat16
    f8 = mybir.dt.float8e4
    Alu = mybir.AluOpType
    Exp = mybir.ActivationFunctionType.Exp

    nc = bacc.Bacc("TRN2", target_bir_lowering=False, debug=False,
                   num_devices=NCORES)

    embt = nc.dram_tensor("embt", [P, NCHUNK, 4, CHUNK], f8,
                          kind="ExternalInput").ap()
    ab = nc.dram_tensor("ab", [2, TPC, P, 4, P], f8,
                        kind="ExternalInput").ap()
    cchl = nc.dram_tensor("cchl", [2, NCHUNK, CHUNK], bf,
                          kind="ExternalInput").ap()
    wsel = nc.dram_tensor("wsel", [P, NT, 2], f32, kind="ExternalInput").ap()
    scale2a = nc.dram_tensor("scale2a", [P, NT, 2], f32,
                             kind="ExternalInput").ap()
    biash = nc.dram_tensor("biash", [P, NT, 2], f32,
                           kind="ExternalInput").ap()
    stab = nc.dram_tensor("stab", [P, NT * 2], f32, kind="ExternalOutput").ap()

    with tile.TileContext(nc) as tc, ExitStack() as ctx:
        consts = ctx.enter_context(tc.tile_pool(name="consts", bufs=1))
        dram = ctx.enter_context(tc.tile_pool(name="dram", bufs=1,
                                              space="DRAM"))
        atp = ctx.enter_context(tc.tile_pool(name="atp", bufs=4))
        ep = ctx.enter_context(tc.tile_pool(name="ep", bufs=4))
        pp = ctx.enter_context(tc.tile_pool(name="pp", bufs=6, space="PSUM"))

        # AllGather the B-sharded pair-row tiles: each core contributes its
        # [2, TPC, P, 4, P] slice; gathered layout is [core, 2, TPC, ...].
        ab_in = dram.tile([2, TPC, P, 4, P], f8)
        ab_all = dram.tile([NCORES, 2, TPC, P, 4, P], f8)
        nc.gpsimd.dma_start(ab_in[:], ab[:])
        nc.gpsimd.collective_compute(
            "AllGather", mybir.AluOpType.bypass,
            replica_groups=[list(range(NCORES))],
            ins=[ab_in.opt()], outs=[ab_all.opt()])

        # whole emb shard lives in SBUF (51.2KB/partition in fp8)
        embt_sb = consts.tile([P, NCHUNK, 4, CHUNK], f8)
        for c in range(NCHUNK):
            nc.sync.dma_start(embt_sb[:, c], embt[:, c])
        cchl_sb = consts.tile([2, NCHUNK, CHUNK], bf)
        nc.sync.dma_start(cchl_sb[:], cchl[:])
        wsel_sb = consts.tile([P, NT, 2], f32)
        nc.sync.dma_start(wsel_sb[:], wsel[:])
        scale2a_sb = consts.tile([P, NT, 2], f32)
        nc.sync.dma_start(scale2a_sb[:], scale2a[:])
        biash_sb = consts.tile([P, NT, 2], f32)
        nc.sync.dma_start(biash_sb[:], biash[:])

        ones2 = consts.tile([2, P], bf)
        nc.gpsimd.memset(ones2[:], 1.0)
        iota_f = consts.tile([P, CHUNK], f32)
        nc.gpsimd.iota(iota_f[:], [[1, CHUNK]], channel_multiplier=0,
                       allow_small_or_imprecise_dtypes=True)
        # negi = -BIG * I, built on device: (j - p == 0) * NEG_BIG
        iota_d = consts.tile([P, P], f32)
        nc.gpsimd.iota(iota_d[:], [[1, P]], channel_multiplier=-1,
                       allow_small_or_imprecise_dtypes=True)
        negi_sb = consts.tile([P, P], bf)
        nc.vector.tensor_scalar(negi_sb[:], iota_d[:], 0.0, NEG_BIG,
                                op0=mybir.AluOpType.is_equal,
                                op1=mybir.AluOpType.mult)
        # one-hot self-suppression rhs, built on device: oh[k, n] = (n == w_k)
        oh_sb = consts.tile([P, NT, 2, CHUNK], bf)
        for t in range(NT):
            for s in (0, 1):
                nc.vector.tensor_scalar(
                    oh_sb[:, t, s], iota_f[:], wsel_sb[:, t, s:s + 1],
                    None, op0=Alu.is_equal)
        stab_sb = consts.tile([P, NT * 2, NCHUNK], f32)
        stab2_sb = consts.tile([P, NT * 2], f32)

        for t in range(NT):
            at = []
            for s in (0, 1):
                a = atp.tile([P, 4, P], f8, tag="at", name=f"at{s}_{t}")
                nc.sync.dma_start(a[:], ab_all[t // TPC, s, t % TPC])
                at.append(a)
            for c in range(NCHUNK):
                w = LAST_W if c == NCHUNK - 1 else CHUNK
                for s in (0, 1):
                    ps = pp.tile([P, CHUNK], f32, tag="ps",
                                 name=f"ps{s}_{t}_{c}")
                    for d in range(4):
                        nc.tensor.matmul(ps[:, :w], lhsT=at[s][:, d, :],
                                         rhs=embt_sb[:, c, d, :w],
                                         start=(d == 0), stop=False)
                    nc.tensor.matmul(ps[:, :w], lhsT=ones2[:],
                                     rhs=cchl_sb[:, c, :w],
                                     start=False, stop=(c != 0))
                    if c == 0:
                        nc.tensor.matmul(ps[:], lhsT=negi_sb[:],
                                         rhs=oh_sb[:, t, s], start=False,
                                         stop=True)
                    sc = ep.tile([P, CHUNK], f32, tag="e",
                                 name=f"e{s}_{t}_{c}")
                    nc.scalar.activation(
                        out=sc[:, :w], in_=ps[:, :w], func=Exp,
                        bias=biash_sb[:, t, s:s + 1],
                        scale=scale2a_sb[:, t, s:s + 1],
                        accum_out=stab_sb[:, t * 2 + s, c:c + 1])

        # fold the 25 per-chunk partial sums into one slot per (tile, side)
        nc.vector.tensor_reduce(out=stab2_sb[:], in_=stab_sb[:],
                                axis=mybir.AxisListType.X, op=Alu.add)
        nc.sync.dma_start(stab[:], stab2_sb[:])

    nc.compile()
    return nc


# --------------------------------------------------------------------------
# host-side combine
# --------------------------------------------------------------------------

def _combine(host, core_results, m0):
    """Returns (result, ok). ok=False if the fixed stabilizer m0 was too far
    from a row's true max (inf or all-zero partials) and a retry with a
    shifted m0 is needed."""
    out = np.zeros(B, dtype=np.float64)
    ok = True
    for s in range(2):
        mu = host["mu_l"] if s == 0 else host["mu_r"]
        sd = host["sd_l"] if s == 0 else host["sd_r"]
        x_self = host["x_self_l"] if s == 0 else host["x_self_r"]
        alpha = LAMB / sd
        Ssum = np.zeros(B, dtype=np.float64)
        for res in core_results:
            S = np.asarray(res["stab"], np.float64).reshape(128, NT, 2)
            if not np.isfinite(S).all():
                ok = False
            Ssum += S[:, :, s].transpose(1, 0).reshape(B)
        # masked entries (all exp(z - m0), z = alpha*(y-mu)+TAU)
        z0 = alpha * (0.0 - mu) + TAU
        zneg = alpha * (-x_self - mu) + TAU
        Ssum += np.where(host["eq"], np.exp(zneg - m0), 2.0 * np.exp(z0 - m0))
        if (Ssum <= 0).any() or not np.isfinite(Ssum).all():
            ok = False
        with np.errstate(divide="ignore"):
            out += m0 + np.log(Ssum)
    return np.float32(out.mean()), ok


# --------------------------------------------------------------------------
# entry point
# --------------------------------------------------------------------------

_CACHED_NC = None


def _enable_jax_compile_cache():
    """Persistent XLA compilation cache: run_bass_kernel_spmd builds a fresh
    jax.jit wrapper per call, so without this every call re-runs the full
    BIR verify/optimise + XLA compile (~1s).  With it, repeat calls load the
    compiled executable from disk."""
    import jax

    try:
        jax.config.update("jax_compilation_cache_dir", "/tmp/jaxcache")
        jax.config.update("jax_persistent_cache_min_compile_time_secs", 0.0)
        jax.config.update("jax_persistent_cache_min_entry_size_bytes", 0)
    except Exception:
        pass


def kernel(pairs, emb, _trace=False, _return_extras=None):
    global _CACHED_NC
    from concourse.bass_utils import run_bass_kernel_spmd

    _enable_jax_compile_cache()

    host = _host_prepare(pairs, emb)
    if _CACHED_NC is None:
        _CACHED_NC = _build_bass()
    nc = _CACHED_NC

    m0 = M0
    result = None
    res = None
    in_maps = None
    for attempt in range(4):
        in_maps = _make_in_maps(host, m0)
        try:
            res = run_bass_kernel_spmd(nc, in_maps,
                                       core_ids=list(range(NCORES)),
                                       trace=_trace)
        except ModuleNotFoundError:
            # no NTFF profile hook in this environment -- run without trace
            res = run_bass_kernel_spmd(nc, in_maps,
                                       core_ids=list(range(NCORES)),
                                       trace=False)
        result, ok = _combine(host, res.results, m0)
        if ok:
            break
        # stabilizer off: inf partials -> raise m0; all-underflow -> lower
        has_inf = any(not np.isfinite(np.asarray(r["stab"])).all()
                      for r in res.results)
        m0 = m0 + 60.0 if has_inf else m0 - 60.0
    if _return_extras is not None:
        _return_extras["exec_time_ns"] = res.exec_time_ns
        _return_extras["bass_results"] = res
        _return_extras["in_maps"] = in_maps
        _return_extras["host"] = host
        _return_extras["m0"] = m0
    return result


if __name__ == "__main__":
    sys.path.insert(0, os.path.dirname(os.path.abspath(__file__)))
    import reference

    inputs = reference.setup_inputs()
    expected = np.asarray(reference.reference(**inputs))
    got = kernel(**{k: np.asarray(v) for k, v in inputs.items()})
    rel = abs(float(got) - float(expected)) / abs(float(expected))
    print("expected:", expected, "got:", got, "rel_err:", rel)


# revision 22
# speedup vs baseline: 1.0344x; 1.0344x over previous
"""Trainium2 Bass kernel for nn_Encoder_Model_15874199126585 (align-loss).

loss = mean_i[ lse_l(i) + lse_r(i) ] where, per side,
  x[i,j] = pos[i] - (||A_i||^2 + ||e_j||^2 - 2 A_i.e_j) + GAMMA
  y      = x * mask          (mask kills cols l_i, r_i)
  lse    = logsumexp(LAMB*(y-mu)/sd + TAU, axis=-1)

Strategy (8 NeuronCores, emb rows N-sharded 12500/core, no collectives):
 * mean/std per row are computed on HOST in f64 closed form (Gram-matrix
   quadratic forms), so the device needs no stats passes or collectives.
 * each core computes its [B, 12800(padded)] slice of x'' = A.e_j + cc_j/2
   (cc_j = -||e_j||^2): per 512-col chunk, 4 fp8 matmuls accumulate the
   dot in PSUM, then a k=2 bf16 matmul (ones  @ [cc_hi; cc_lo]) adds the
   per-column constant, so no DVE pass and no replicated cc upload.
 * the "self" column (j == own index, value pos+GAMMA, which would dominate
   the softmax) is killed inside PSUM by one more accumulating matmul:
   (-1e30*I).T @ onehot, where the host permutation placed every column that
   can ever be a self column into chunk 0 ("hot block").  The onehot rhs and
   the -1e30*I weights are built ON DEVICE from a 16KB index upload
   (iota + is_equal), not uploaded.
 * the gathered pair-row tiles (lt/rt) are NOT replicated to all 8 cores:
   each core uploads 2 of the 16 row tiles (262KB) and a device AllGather
   (DRAM bounce buffers, replica group 0-7) reconstructs the full set.
 * because rows are exactly normalized, z = LAMB*(x-mu)/sd + TAU lies in a
   known narrow band, so a FIXED stabilizer M0 replaces the usual row-max:
   one fused ACT pass per chunk computes exp(psum*(2a) + bias) with bias =
   a*(rc-mu)+TAU-M0 precomputed on host (rc = pos - ||A||^2 + GAMMA), and
   its accum_out gives the chunk row-sum for free.
 * device emits per-(row, tile, side, chunk) partial sums S; host does the
   log-sum-exp combine in f64 and adds the analytic contribution of the
   masked-out entries.

All tensor inputs ride to the device in fp8e4m3 (emb values are N(0,1),
max |v| ~ 5, well inside e4m3 range); the exact per-row normalization and
the final f64 combine keep the end-to-end relative error ~1e-3, far inside
the 2e-2 gate, while cutting per-call PJRT upload from ~225MB to ~55MB --
the axon-tunnel transfer is what dominates wall time, not device compute
(~1ms).  A persistent XLA compilation cache removes the ~1s/call re-verify
that run_bass_kernel_spmd's fresh-jit-per-call pattern otherwise incurs.
"""

import os
import sys
from contextlib import ExitStack

import numpy as np

sys.path.insert(0, "/opt/trn_rl_repo")

import ml_dtypes

NODE = 100000
DIM = 512
B = 2048
GAMMA, LAMB, TAU = 3.0, 20.0, 8.0
NCORES = 8
CHUNK = 512
NCHUNK = 25
NS_PAD = NCHUNK * CHUNK          # 12800 DRAM-layout columns per core
LAST_W = 256                     # last chunk is trimmed to 256 columns
NS_USED = (NCHUNK - 1) * CHUNK + LAST_W  # 12544 columns actually computed
NS_REAL = NODE // NCORES         # 12500
HOT = 512                        # hot block = chunk 0 (all possible self cols)
NT = B // 128                    # 16 row tiles
NEG_BIG = -1.0e30
M0 = 100.0                       # fixed logsumexp stabilizer (z in [~84, ~110])
TPC = NT // NCORES               # pair row tiles uploaded per core (AllGathered)

FP8 = ml_dtypes.float8_e4m3      # TRN2 float8e4


# --------------------------------------------------------------------------
# host-side preparation
# --------------------------------------------------------------------------

def _host_prepare(pairs, emb):
    pairs = np.asarray(pairs)
    emb = np.asarray(emb, dtype=np.float32)
    l = pairs[:, 0].astype(np.int64)
    r = pairs[:, 1].astype(np.int64)
    emb64 = emb.astype(np.float64)

    l_emb = emb[l]
    r_emb = emb[r]
    l64, r64 = emb64[l], emb64[r]

    emb_sq64 = np.sum(emb64 * emb64, axis=1)
    pos64 = np.sum((l64 - r64) ** 2, axis=1)
    a_sq64 = emb_sq64[l]
    b_sq64 = emb_sq64[r]
    cc64 = -emb_sq64

    rc_l = pos64 - a_sq64 + GAMMA
    rc_r = pos64 - b_sq64 + GAMMA

    s_vec = emb64.sum(axis=0)
    w_vec = (emb64 * cc64[:, None]).sum(axis=0)
    C1 = cc64.sum()
    C2 = (cc64 * cc64).sum()
    try:
        from scipy.linalg.blas import dsyrk
        G = dsyrk(1.0, emb64, trans=1)       # upper triangle of emb64.T@emb64
        G = np.triu(G) + np.triu(G, 1).T
    except Exception:
        G = emb64.T @ emb64

    def side_stats(A64, rc):
        As = A64 @ s_vec
        Aw = A64 @ w_vec
        qf = np.einsum("bd,bd->b", A64 @ G, A64)
        S1 = 2.0 * As + NODE * rc + C1
        S2 = (4.0 * qf + 4.0 * Aw + 4.0 * rc * As + NODE * rc * rc
              + 2.0 * rc * C1 + C2)
        return S1, S2

    S1_l, S2_l = side_stats(l64, rc_l)
    S1_r, S2_r = side_stats(r64, rc_r)

    dot_lr = np.einsum("bd,bd->b", l64, r64)
    x_self_l = 2.0 * a_sq64 + rc_l + cc64[l]
    x_cross_l = 2.0 * dot_lr + rc_l + cc64[r]
    x_self_r = 2.0 * b_sq64 + rc_r + cc64[r]
    x_cross_r = 2.0 * dot_lr + rc_r + cc64[l]

    eq = l == r

    def masked_stats(S1, S2, x_self, x_cross):
        S1m = np.where(eq, S1 - 2.0 * x_self, S1 - x_self - x_cross)
        S2m = np.where(eq, S2, S2 - x_self ** 2 - x_cross ** 2)
        mu = S1m / NODE
        var = S2m / NODE - mu * mu
        sd = np.sqrt(var)
        return mu, sd

    mu_l, sd_l = masked_stats(S1_l, S2_l, x_self_l, x_cross_l)
    mu_r, sd_r = masked_stats(S1_r, S2_r, x_self_r, x_cross_r)

    # core assignment: every value appearing in pairs goes into some core's
    # 512-column hot block (front of its local column range)
    hot = np.unique(np.concatenate([l, r]))
    hot_per_core = [hot[c::NCORES] for c in range(NCORES)]
    for c in range(NCORES):
        assert len(hot_per_core[c]) <= HOT - 1, (c, len(hot_per_core[c]))
    cold_mask = np.ones(NODE, dtype=bool)
    cold_mask[hot] = False
    cold = np.nonzero(cold_mask)[0]

    bf16 = ml_dtypes.bfloat16
    cores = []
    off = 0
    for c in range(NCORES):
        nh = len(hot_per_core[c])
        need = NS_REAL - nh
        cold_c = cold[off:off + need]
        off += need
        colmap = np.full(NS_USED, -1, dtype=np.int64)
        colmap[:nh] = hot_per_core[c]
        assert HOT + need <= NS_USED
        colmap[HOT:HOT + need] = cold_c
        valid = colmap >= 0

        embT = np.zeros((DIM, NS_USED), dtype=np.float32)
        embT[:, valid] = emb[colmap[valid]].T
        cch64 = np.full(NS_USED, NEG_BIG / 2, dtype=np.float64)
        cch64[valid] = cc64[colmap[valid]] / 2.0

        g2loc = {int(colmap[j]): j for j in range(nh)}
        padcol = HOT - 1
        assert colmap[padcol] == -1
        w_l = np.array([g2loc.get(int(v), padcol) for v in l], dtype=np.int64)
        w_r = np.array([g2loc.get(int(v), padcol) for v in r], dtype=np.int64)

        # device input layouts
        # embt: flat [128(k), sum_c 4*w_c] fp8; per chunk the block is
        # [k, 4(d), w_c] flattened, chunk widths 24x512 + 1x256 (no padding)
        embT8 = embT.astype(FP8)
        blocks = []
        for ci in range(NCHUNK):
            wc = LAST_W if ci == NCHUNK - 1 else CHUNK
            blk = embT8[:, ci * CHUNK:ci * CHUNK + wc]
            blocks.append(blk.reshape(4, 128, wc).transpose(1, 0, 2)
                          .reshape(128, 4 * wc))
        embt_dev = np.ascontiguousarray(np.concatenate(blocks, axis=1))
        # cc/2 split hi/lo in bf16: [2, NS_USED]
        hi = cch64.astype(bf16)
        lo = (cch64 - hi.astype(np.float64)).astype(bf16)
        cchl_dev = np.ascontiguousarray(np.stack([hi, lo], axis=0))
        # self-suppression one-hot column index per (row-in-tile, tile, side)
        wsel = np.stack([w_l.reshape(NT, 128), w_r.reshape(NT, 128)], axis=-1)
        wsel_dev = np.ascontiguousarray(
            wsel.transpose(1, 0, 2).astype(np.float32))     # [128, NT, 2]
        cores.append(dict(embt=embt_dev, cchl=cchl_dev, wsel=wsel_dev))
    assert off == len(cold)

    # pair-row tiles, B-sharded across cores: core c uploads row tiles
    # {2c, 2c+1} of both sides and the device AllGathers the full set.
    def tile_A(A):
        # A [B, D] f32 -> [NT, 128(k), 4(d), 128(m)] fp8 of A^T
        At = A.T.astype(FP8)                      # [D, B]
        return np.ascontiguousarray(
            At.reshape(4, 128, NT, 128).transpose(2, 1, 0, 3))

    lt_dev = tile_A(l_emb)
    rt_dev = tile_A(r_emb)
    TPC = NT // NCORES               # row tiles per core
    ab_shards = []
    for c in range(NCORES):
        sh = np.stack([lt_dev[c * TPC:(c + 1) * TPC],
                       rt_dev[c * TPC:(c + 1) * TPC]], axis=0)
        ab_shards.append(np.ascontiguousarray(sh))  # [2, TPC, 128, 4, 128]

    alpha_l = LAMB / sd_l
    alpha_r = LAMB / sd_r
    scale2a = np.stack([2.0 * alpha_l, 2.0 * alpha_r], axis=-1)
    biash0 = np.stack([alpha_l * (rc_l - mu_l) + TAU,
                       alpha_r * (rc_r - mu_r) + TAU], axis=-1)
    scale2a_dev = np.ascontiguousarray(
        scale2a.reshape(NT, 128, 2).transpose(1, 0, 2)).astype(np.float32)
    biash0_dev = np.ascontiguousarray(
        biash0.reshape(NT, 128, 2).transpose(1, 0, 2))
    host = dict(
        eq=eq, mu_l=mu_l, sd_l=sd_l, mu_r=mu_r, sd_r=sd_r,
        x_self_l=x_self_l, x_self_r=x_self_r,
        cores=cores, ab_shards=ab_shards,
        scale2a=scale2a_dev, biash0=biash0_dev,
    )
    return host


def _make_in_maps(host, m0):
    biash = (host["biash0"] - m0).astype(np.float32)
    in_maps = []
    for c in range(NCORES):
        core = host["cores"][c]
        in_maps.append(dict(
            embt=core["embt"], ab=host["ab_shards"][c],
            cchl=core["cchl"], wsel=core["wsel"],
            scale2a=host["scale2a"], biash=biash,
        ))
    return in_maps


# --------------------------------------------------------------------------
# bass kernel
# --------------------------------------------------------------------------

def _build_bass():
    import concourse.mybir as mybir
    import concourse.tile as tile
    from concourse import bacc

    P = 128
    f32 = mybir.dt.float32
    bf = mybir.dt.bfloat16
    f8 = mybir.dt.float8e4
    Alu = mybir.AluOpType
    Exp = mybir.ActivationFunctionType.Exp

    nc = bacc.Bacc("TRN2", target_bir_lowering=False, debug=False,
                   num_devices=NCORES)

    EMBT_COLS = 4 * NS_USED
    embt = nc.dram_tensor("embt", [P, EMBT_COLS], f8,
                          kind="ExternalInput").ap()
    ab = nc.dram_tensor("ab", [2, TPC, P, 4, P], f8,
                        kind="ExternalInput").ap()
    cchl = nc.dram_tensor("cchl", [2, NS_USED], bf,
                          kind="ExternalInput").ap()
    wsel = nc.dram_tensor("wsel", [P, NT, 2], f32, kind="ExternalInput").ap()
    scale2a = nc.dram_tensor("scale2a", [P, NT, 2], f32,
                             kind="ExternalInput").ap()
    biash = nc.dram_tensor("biash", [P, NT, 2], f32,
                           kind="ExternalInput").ap()
    stab = nc.dram_tensor("stab", [P, NT * 2], f32, kind="ExternalOutput").ap()

    with tile.TileContext(nc) as tc, ExitStack() as ctx:
        consts = ctx.enter_context(tc.tile_pool(name="consts", bufs=1))
        dram = ctx.enter_context(tc.tile_pool(name="dram", bufs=1,
                                              space="DRAM"))
        atp = ctx.enter_context(tc.tile_pool(name="atp", bufs=4))
        ep = ctx.enter_context(tc.tile_pool(name="ep", bufs=4))
        pp = ctx.enter_context(tc.tile_pool(name="pp", bufs=6, space="PSUM"))

        # AllGather the B-sharded pair-row tiles: each core contributes its
        # [2, TPC, P, 4, P] slice; gathered layout is [core, 2, TPC, ...].
        ab_in = dram.tile([2, TPC, P, 4, P], f8)
        ab_all = dram.tile([NCORES, 2, TPC, P, 4, P], f8)
        nc.gpsimd.dma_start(ab_in[:], ab[:])
        nc.gpsimd.collective_compute(
            "AllGather", mybir.AluOpType.bypass,
            replica_groups=[list(range(NCORES))],
            ins=[ab_in.opt()], outs=[ab_all.opt()])

        # whole emb shard lives in SBUF (50.2KB/partition in fp8)
        embt_sb = consts.tile([P, EMBT_COLS], f8)
        for c in range(NCHUNK):
            w = LAST_W if c == NCHUNK - 1 else CHUNK
            boff = c * 4 * CHUNK
            nc.sync.dma_start(embt_sb[:, boff:boff + 4 * w],
                              embt[:, boff:boff + 4 * w])
        cchl_sb = consts.tile([2, NS_USED], bf)
        nc.sync.dma_start(cchl_sb[:], cchl[:])
        wsel_sb = consts.tile([P, NT, 2], f32)
        nc.sync.dma_start(wsel_sb[:], wsel[:])
        scale2a_sb = consts.tile([P, NT, 2], f32)
        nc.sync.dma_start(scale2a_sb[:], scale2a[:])
        biash_sb = consts.tile([P, NT, 2], f32)
        nc.sync.dma_start(biash_sb[:], biash[:])

        ones2 = consts.tile([2, P], bf)
        nc.gpsimd.memset(ones2[:], 1.0)
        iota_f = consts.tile([P, CHUNK], f32)
        nc.gpsimd.iota(iota_f[:], [[1, CHUNK]], channel_multiplier=0,
                       allow_small_or_imprecise_dtypes=True)
        # negi = -BIG * I, built on device: (j - p == 0) * NEG_BIG
        iota_d = consts.tile([P, P], f32)
        nc.gpsimd.iota(iota_d[:], [[1, P]], channel_multiplier=-1,
                       allow_small_or_imprecise_dtypes=True)
        negi_sb = consts.tile([P, P], bf)
        nc.vector.tensor_scalar(negi_sb[:], iota_d[:], 0.0, NEG_BIG,
                                op0=mybir.AluOpType.is_equal,
                                op1=mybir.AluOpType.mult)
        # one-hot self-suppression rhs, built on device: oh[k, n] = (n == w_k)
        oh_sb = consts.tile([P, NT, 2, CHUNK], bf)
        for t in range(NT):
            for s in (0, 1):
                nc.vector.tensor_scalar(
                    oh_sb[:, t, s], iota_f[:], wsel_sb[:, t, s:s + 1],
                    None, op0=Alu.is_equal)
        stab_sb = consts.tile([P, NT * 2, NCHUNK], f32)
        stab2_sb = consts.tile([P, NT * 2], f32)

        for t in range(NT):
            at = []
            for s in (0, 1):
                a = atp.tile([P, 4, P], f8, tag="at", name=f"at{s}_{t}")
                nc.sync.dma_start(a[:], ab_all[t // TPC, s, t % TPC])
                at.append(a)
            for c in range(NCHUNK):
                w = LAST_W if c == NCHUNK - 1 else CHUNK
                boff = c * 4 * CHUNK
                for s in (0, 1):
                    ps = pp.tile([P, CHUNK], f32, tag="ps",
                                 name=f"ps{s}_{t}_{c}")
                    for d in range(4):
                        nc.tensor.matmul(
                            ps[:, :w], lhsT=at[s][:, d, :],
                            rhs=embt_sb[:, boff + d * w:boff + (d + 1) * w],
                            start=(d == 0), stop=False)
                    nc.tensor.matmul(ps[:, :w], lhsT=ones2[:],
                                     rhs=cchl_sb[:, c * CHUNK:c * CHUNK + w],
                                     start=False, stop=(c != 0))
                    if c == 0:
                        nc.tensor.matmul(ps[:], lhsT=negi_sb[:],
                                         rhs=oh_sb[:, t, s], start=False,
                                         stop=True)
                    sc = ep.tile([P, CHUNK], f32, tag="e",
                                 name=f"e{s}_{t}_{c}")
                    nc.scalar.activation(
                        out=sc[:, :w], in_=ps[:, :w], func=Exp,
                        bias=biash_sb[:, t, s:s + 1],
                        scale=scale2a_sb[:, t, s:s + 1],
                        accum_out=stab_sb[:, t * 2 + s, c:c + 1])

        # fold the 25 per-chunk partial sums into one slot per (tile, side)
        nc.vector.tensor_reduce(out=stab2_sb[:], in_=stab_sb[:],
                                axis=mybir.AxisListType.X, op=Alu.add)
        nc.sync.dma_start(stab[:], stab2_sb[:])

    nc.compile()
    return nc


# --------------------------------------------------------------------------
# host-side combine
# --------------------------------------------------------------------------

def _combine(host, core_results, m0):
    """Returns (result, ok). ok=False if the fixed stabilizer m0 was too far
    from a row's true max (inf or all-zero partials) and a retry with a
    shifted m0 is needed."""
    out = np.zeros(B, dtype=np.float64)
    ok = True
    for s in range(2):
        mu = host["mu_l"] if s == 0 else host["mu_r"]
        sd = host["sd_l"] if s == 0 else host["sd_r"]
        x_self = host["x_self_l"] if s == 0 else host["x_self_r"]
        alpha = LAMB / sd
        Ssum = np.zeros(B, dtype=np.float64)
        for res in core_results:
            S = np.asarray(res["stab"], np.float64).reshape(128, NT, 2)
            if not np.isfinite(S).all():
                ok = False
            Ssum += S[:, :, s].transpose(1, 0).reshape(B)
        # masked entries (all exp(z - m0), z = alpha*(y-mu)+TAU)
        z0 = alpha * (0.0 - mu) + TAU
        zneg = alpha * (-x_self - mu) + TAU
        Ssum += np.where(host["eq"], np.exp(zneg - m0), 2.0 * np.exp(z0 - m0))
        if (Ssum <= 0).any() or not np.isfinite(Ssum).all():
            ok = False
        with np.errstate(divide="ignore"):
            out += m0 + np.log(Ssum)
    return np.float32(out.mean()), ok


# --------------------------------------------------------------------------
# entry point
# --------------------------------------------------------------------------

_CACHED_NC = None


def _enable_jax_compile_cache():
    """Persistent XLA compilation cache: run_bass_kernel_spmd builds a fresh
    jax.jit wrapper per call, so without this every call re-runs the full
    BIR verify/optimise + XLA compile (~1s).  With it, repeat calls load the
    compiled executable from disk."""
    import jax

    try:
        jax.config.update("jax_compilation_cache_dir", "/tmp/jaxcache")
        jax.config.update("jax_persistent_cache_min_compile_time_secs", 0.0)
        jax.config.update("jax_persistent_cache_min_entry_size_bytes", 0)
    except Exception:
        pass


def kernel(pairs, emb, _trace=False, _return_extras=None):
    global _CACHED_NC
    from concourse.bass_utils import run_bass_kernel_spmd

    _enable_jax_compile_cache()

    host = _host_prepare(pairs, emb)
    if _CACHED_NC is None:
        _CACHED_NC = _build_bass()
    nc = _CACHED_NC

    m0 = M0
    result = None
    res = None
    in_maps = None
    for attempt in range(4):
        in_maps = _make_in_maps(host, m0)
        try:
            res = run_bass_kernel_spmd(nc, in_maps,
                                       core_ids=list(range(NCORES)),
                                       trace=_trace)
        except ModuleNotFoundError:
            # no NTFF profile hook in this environment -- run without trace
            res = run_bass_kernel_spmd(nc, in_maps,
                                       core_ids=list(range(NCORES)),
                                       trace=False)
        result, ok = _combine(host, res.results, m0)
        if ok:
            break
        # stabilizer off: inf partials -> raise m0; all-underflow -> lower
        has_inf = any(not np.isfinite(np.asarray(r["stab"])).all()
                      for r in res.results)
        m0 = m0 + 60.0 if has_inf else m0 - 60.0
    if _return_extras is not None:
        _return_extras["exec_time_ns"] = res.exec_time_ns
        _return_extras["bass_results"] = res
        _return_extras["in_maps"] = in_maps
        _return_extras["host"] = host
        _return_extras["m0"] = m0
    return result


if __name__ == "__main__":
    sys.path.insert(0, os.path.dirname(os.path.abspath(__file__)))
    import reference

    inputs = reference.setup_inputs()
    expected = np.asarray(reference.reference(**inputs))
    got = kernel(**{k: np.asarray(v) for k, v in inputs.items()})
    rel = abs(float(got) - float(expected)) / abs(float(expected))
    print("expected:", expected, "got:", got, "rel_err:", rel)
